# revision 1
# baseline (speedup 1.0000x reference)
"""DD-RoPE kernel for 8x TRN2 NeuronCores.

Reference computation (B=4, T=4096, D=2048, P=256):
    deltas = einsum('btd,pd->btp', x, W) + b     # (B, T, P)
    angles = cumsum(deltas, axis=1)
    out = concat([x1*cos(a) - x2*sin(a), x2*cos(a) + x1*sin(a), x[..., 512:]], -1)

Sharding: 8 shards = 4 batches x 2 T-halves (2048 each), data-parallel.
The cumsum is handled with host-computed fp64 "block bases": the exact
cumulative angle at every 128-step boundary (one [256, 16] vector set per
shard, computed from 128-step block sums of x in one pass). Each on-device
prefix scan then only spans 128 steps, so per-delta rounding error from the
reduced-precision matmul amplifies by at most sqrt(128), and there is no
cross-core (or even cross-block) dependency at all.

Per-core dataflow (all tensors in [feature-partition, time-free] layout):
    xf fp16 = fp16(x_shard^T), one dense 2MB DMA per 512-step time block
              (d-chunks side by side in the free dim of one SBUF tile)
    deltas^T = wh_f16^T @ xf + wlo_bf16^T @ xf + b_hi + b_lo
               (PE: 2 passes, mixed-dtype second pass, fp32 PSUM;
                split precision is needed because the cumsum amplifies
                per-delta error by sqrt(block))
    angles^T = per-128 prefix scans of deltas^T, initial = host base
    range-reduce in turns (magic-number rounding), sin/cos via ScalarE Sin
    rotation on DVE (o1) + GpSimd (o2), written into one output tile per
    time block -> one dense 1MB DMA out
    host reassembles the rotated half; passthrough cols copied on host.
"""

import sys

if "/opt/trn_rl_repo" not in sys.path:
    sys.path.insert(0, "/opt/trn_rl_repo")

from contextlib import ExitStack

import ml_dtypes
import numpy as np

import concourse.bacc as bacc
import concourse.bass as bass
import concourse.mybir as mybir
import concourse.tile as tile
from concourse.bass_utils import run_bass_kernel_spmd

F32 = mybir.dt.float32
F16 = mybir.dt.float16
BF16 = mybir.dt.bfloat16
ADD = mybir.AluOpType.add
SUB = mybir.AluOpType.subtract
IDENT = mybir.ActivationFunctionType.Identity
SIN = mybir.ActivationFunctionType.Sin

D = 2048          # input feature dim (contraction)
P = 256           # delta-pairs dim
ROT = 2 * P       # rotated columns (512)
TL = 2048         # time steps per shard
TB = 512          # time block (one PSUM bank at fp32)
SB = 128          # scan block (base injection granularity)
NT = TL // TB     # time blocks per shard (4)
NBK = TL // SB    # scan blocks per shard (16)
KC = D // 128     # contraction chunks (16)
N_CORES = 8

MAGIC = 12582912.0          # 1.5 * 2**23: fp32 round-to-int magic constant
SCALE_2PI = 6.28310         # slightly < 2*pi so Sin args stay inside [-pi, pi]
COS_BIAS = 1.5707964        # ~pi/2 (fp32)
NP_BF16 = np.dtype(ml_dtypes.bfloat16)


def build_program(tl: int = TL) -> bass.Bass:
    nt = tl // TB
    nbk = tl // SB
    nc = bacc.Bacc("TRN2", target_bir_lowering=False, debug=False)

    # Host-pre-tiled inputs: every DMA below reads one dense DRAM block.
    # xf row block tb: [128, KC*TB] fp16 (d-chunks along the free dim)
    xf = nc.dram_tensor("xf", [nt * 128, KC * TB], F16,
                        kind="ExternalInput").ap()
    wh = nc.dram_tensor("wh", [128, KC * P], F16, kind="ExternalInput").ap()
    wl = nc.dram_tensor("wl", [128, KC * P], BF16, kind="ExternalInput").ap()
    bv = nc.dram_tensor("bv", [1, 2 * P], BF16, kind="ExternalInput").ap()
    # per-128-block angle bases (turns), [P, nbk] fp32
    bs = nc.dram_tensor("bs", [P, nbk], F32, kind="ExternalInput").ap()
    # out row block tb: [128, 4*TB] f32 (quadrants o1h0|o1h1|o2h0|o2h1)
    outT = nc.dram_tensor("outT", [nt * 128, 4 * TB], F32,
                          kind="ExternalOutput").ap()

    with tile.TileContext(nc) as tc, ExitStack() as ctx:
        const_pool = ctx.enter_context(tc.tile_pool(name="const", bufs=1))
        w_pool = ctx.enter_context(tc.tile_pool(name="w", bufs=1))
        x_pool = ctx.enter_context(tc.tile_pool(name="x", bufs=2))
        psum_pool = ctx.enter_context(tc.tile_pool(name="psum", bufs=4, space="PSUM"))
        ang_pool = ctx.enter_context(tc.tile_pool(name="ang", bufs=2))
        trig_pool = ctx.enter_context(tc.tile_pool(name="trig", bufs=2))
        rot_pool = ctx.enter_context(tc.tile_pool(name="rot", bufs=2))
        out_pool = ctx.enter_context(tc.tile_pool(name="out", bufs=2))

        # Weights (stationary): one dense DMA per precision level
        wh_sb = w_pool.tile([128, KC * P], F16, tag="wh")
        nc.sync.dma_start(wh_sb[:], wh[:])
        wl_sb = w_pool.tile([128, KC * P], BF16, tag="wl")
        nc.sync.dma_start(wl_sb[:], wl[:])
        bs_sb = const_pool.tile([128, 2 * nbk], F32, tag="bs")
        nc.sync.dma_start(bs_sb[:, 0:nbk], bs[0:128, :])
        nc.sync.dma_start(bs_sb[:, nbk:2 * nbk], bs[128:256, :])
        bv_sb = const_pool.tile([1, 2 * P], BF16, tag="bv")
        nc.sync.dma_start(bv_sb[:], bv[:])
        ones_sb = const_pool.tile([1, TB], BF16, tag="ones")
        nc.gpsimd.memset(ones_sb[:], 1.0)
        zeros_sb = const_pool.tile([128, SB], F32, tag="zeros")
        nc.gpsimd.memset(zeros_sb[:], 0.0)
        magic_sb = const_pool.tile([128, 1], F32, tag="magic")
        nc.gpsimd.memset(magic_sb[:], MAGIC)
        negq_sb = const_pool.tile([128, 1], F32, tag="negq")
        nc.gpsimd.memset(negq_sb[:], -0.25)
        cosb_sb = const_pool.tile([128, 1], F32, tag="cosb")
        nc.gpsimd.memset(cosb_sb[:], COS_BIAS)

        for tb in range(nt):
            # one dense 2MB x DMA per time block
            xall = x_pool.tile([128, KC * TB], F16, tag="xall")
            nc.sync.dma_start(xall[:], xf[tb * 128:(tb + 1) * 128, :])
            oall = out_pool.tile([128, 4 * TB], F32, tag="oall")

            for h in range(2):
                # deltas^T (+bias) in PSUM: b_hi + b_lo + wh@xf + wl@xf
                dp = psum_pool.tile([128, TB], F32, tag="dp")
                nc.tensor.matmul(dp[:], bv_sb[0:1, h * 128:(h + 1) * 128],
                                 ones_sb[:], start=True, stop=False)
                nc.tensor.matmul(dp[:], bv_sb[0:1, P + h * 128:P + (h + 1) * 128],
                                 ones_sb[:], start=False, stop=False)
                for d in range(KC):
                    ws = slice(d * P + h * 128, d * P + (h + 1) * 128)
                    xs = slice(d * TB, (d + 1) * TB)
                    nc.tensor.matmul(dp[:], wh_sb[:, ws], xall[:, xs],
                                     start=False, stop=False)
                    nc.tensor.matmul(dp[:], wl_sb[:, ws], xall[:, xs],
                                     start=False, stop=(d == KC - 1))

                # cumulative angle (turns): independent per-128 scans with
                # host-computed initial bases
                ang = ang_pool.tile([128, TB], F32, tag=f"ang{h}")
                for k in range(TB // SB):
                    kb = tb * (TB // SB) + k
                    cs = slice(k * SB, (k + 1) * SB)
                    nc.vector.tensor_tensor_scan(
                        ang[:, cs], dp[:, cs], zeros_sb[:],
                        initial=bs_sb[:, h * nbk + kb:h * nbk + kb + 1],
                        op0=ADD, op1=ADD)

                # range reduction (turns): rs = y - round(y) in [-0.5, 0.5]
                a_s = trig_pool.tile([128, TB], F32, tag="a_s")
                nc.scalar.activation(a_s[:], ang[:], IDENT,
                                     bias=magic_sb[:], scale=-1.0)
                rs = trig_pool.tile([128, TB], F32, tag="rs")
                nc.vector.scalar_tensor_tensor(rs[:], a_s[:], MAGIC, ang[:],
                                               op0=SUB, op1=ADD)
                sin_t = trig_pool.tile([128, TB], F32, tag="sin")
                nc.scalar.activation(sin_t[:], rs[:], SIN, scale=SCALE_2PI)

                # rc = y - round(y + 0.25) in [-0.75, 0.25];
                # cos(2pi*y) = sin(2pi*rc + pi/2)
                b1 = trig_pool.tile([128, TB], F32, tag="b1")
                nc.scalar.activation(b1[:], ang[:], IDENT,
                                     bias=negq_sb[:], scale=-1.0)
                ac = trig_pool.tile([128, TB], F32, tag="ac")
                nc.scalar.activation(ac[:], b1[:], IDENT, bias=magic_sb[:])
                rc = trig_pool.tile([128, TB], F32, tag="rc")
                nc.vector.scalar_tensor_tensor(rc[:], ac[:], MAGIC, ang[:],
                                               op0=SUB, op1=ADD)
                cos_t = trig_pool.tile([128, TB], F32, tag="cos")
                nc.scalar.activation(cos_t[:], rc[:], SIN,
                                     scale=SCALE_2PI, bias=cosb_sb[:])

                # rotation: x1^T = d-chunk h, x2^T = d-chunk 2+h of xall.
                # o1 on DVE, o2 on the otherwise idle GpSimd.
                x1s = xall[:, h * TB:(h + 1) * TB]
                x2s = xall[:, (2 + h) * TB:(3 + h) * TB]
                t1 = rot_pool.tile([128, TB], F32, tag="t1")
                nc.vector.tensor_mul(t1[:], x1s, cos_t[:])
                t2 = rot_pool.tile([128, TB], F32, tag="t2")
                nc.vector.tensor_mul(t2[:], x2s, sin_t[:])
                o1 = oall[:, h * TB:(h + 1) * TB]
                nc.vector.tensor_sub(o1, t1[:], t2[:])
                t3 = rot_pool.tile([128, TB], F32, tag="t3")
                nc.gpsimd.tensor_mul(t3[:], x2s, cos_t[:])
                t4 = rot_pool.tile([128, TB], F32, tag="t4")
                nc.gpsimd.tensor_mul(t4[:], x1s, sin_t[:])
                o2 = oall[:, (2 + h) * TB:(3 + h) * TB]
                nc.gpsimd.tensor_add(o2, t3[:], t4[:])

            nc.sync.dma_start(outT[tb * 128:(tb + 1) * 128, :], oall[:])

    nc.compile()
    return nc


_NC_CACHE: dict = {}


def _get_nc():
    if "nc" not in _NC_CACHE:
        _NC_CACHE["nc"] = build_program()
    return _NC_CACHE["nc"]


def _tile_x(xt16: np.ndarray, nt: int) -> np.ndarray:
    """[D, tl] fp16 -> [nt*128, KC*TB]: row block tb, d-chunks along free."""
    tl = xt16.shape[1]
    a = xt16.reshape(KC, 128, tl // TB, TB).transpose(2, 1, 0, 3)
    return np.ascontiguousarray(a.reshape((tl // TB) * 128, KC * TB))


def prepare_weights(W: np.ndarray, b: np.ndarray):
    inv2pi = 1.0 / (2.0 * np.pi)
    Wt = W.astype(np.float64).T * inv2pi                           # [D, P]
    bt = b.astype(np.float64) * inv2pi                             # [P]
    whf = Wt.astype(np.float16)
    wlo = (Wt - whf.astype(np.float64)).astype(NP_BF16)
    # [D, P] -> [128, KC*P] with d-chunks along free dim
    wh_in = np.ascontiguousarray(
        whf.reshape(KC, 128, P).transpose(1, 0, 2).reshape(128, KC * P))
    wl_in = np.ascontiguousarray(
        wlo.reshape(KC, 128, P).transpose(1, 0, 2).reshape(128, KC * P))
    bh = bt.astype(NP_BF16)
    bl = (bt - bh.astype(np.float64)).astype(NP_BF16)
    bv_in = np.ascontiguousarray(np.concatenate([bh, bl])[None, :])
    # device-effective weights/bias for the host base computation
    w_eff = whf.astype(np.float64) + wlo.astype(np.float64)
    b_eff = bh.astype(np.float64) + bl.astype(np.float64)
    return wh_in, wl_in, bv_in, w_eff, b_eff


def make_in_maps(x: np.ndarray, W: np.ndarray, b: np.ndarray):
    B = x.shape[0]
    wh_in, wl_in, bv_in, w_eff, b_eff = prepare_weights(W, b)

    # fp64 cumulative angle at every 128-step boundary, per batch (in turns):
    # one pass of 128-block sums over x, then a small [32, D] @ [D, P] matmul
    T = x.shape[1]
    nblk = T // SB                                                  # 32
    xblk = x.reshape(B, nblk, SB, D).sum(axis=2, dtype=np.float64)  # [B, 32, D]
    dblk = xblk @ w_eff + SB * b_eff                                # [B, 32, P]
    bases = np.zeros((B, nblk, P))
    np.cumsum(dblk[:, :-1], axis=1, out=bases[:, 1:])               # exclusive

    in_maps = []
    for c in range(N_CORES):
        bb, hh = c // 2, c % 2
        xt16 = x[bb, hh * TL:(hh + 1) * TL, :].T.astype(np.float16)
        bs_in = bases[bb, hh * NBK:(hh + 1) * NBK].T                # [P, NBK]
        in_maps.append({
            "xf": _tile_x(xt16, NT),
            "wh": wh_in,
            "wl": wl_in,
            "bv": bv_in,
            "bs": np.ascontiguousarray(bs_in.astype(np.float32)),
        })
    return in_maps


def assemble_output(x: np.ndarray, results) -> np.ndarray:
    B, T, Din = x.shape
    out = np.empty((B, T, Din), np.float32)
    out[:, :, ROT:] = x[:, :, ROT:]
    for c in range(N_CORES):
        bb, hh = c // 2, c % 2
        r = results[c]["outT"].reshape(NT, 128, 4, TB)
        # [tb, pp, q(oi,h), u] -> [t_local(tb,u), p(oi,h,pp)]
        blk = r.transpose(0, 3, 2, 1).reshape(TL, ROT)
        out[bb, hh * TL:(hh + 1) * TL, :ROT] = blk
    return out


def kernel(x: np.ndarray, W: np.ndarray, b: np.ndarray) -> np.ndarray:
    nc = _get_nc()
    in_maps = make_in_maps(x, W, b)
    res = run_bass_kernel_spmd(nc, in_maps, list(range(N_CORES)))
    return assemble_output(x, res.results)



# revision 4
# speedup vs baseline: 1.2210x; 1.2210x over previous
"""DD-RoPE kernel for 8x TRN2 NeuronCores.

Reference computation (B=4, T=4096, D=2048, P=256):
    deltas = einsum('btd,pd->btp', x, W) + b     # (B, T, P)
    angles = cumsum(deltas, axis=1)
    out = concat([x1*cos(a) - x2*sin(a), x2*cos(a) + x1*sin(a), x[..., 512:]], -1)

Sharding: 8 shards = 4 batches x 2 T-halves (2048 each), data-parallel.

Numerics: single fp16 matmul pass (W fp16, x fp16, fp32 PSUM). Host sends the
exact fp64 cumulative angle (from the TRUE weights) at every 64-step boundary,
so all quantization error is a <=64-step random walk (sim rel-err ~5e-3 vs
2e-2 budget). Bias + per-block bases enter through one extra "setup" matmul
per time block: stationary rows [b_hi, b_lo, (base_hi_k, base_lo_k)*] (bf16
pairs = exact f32 injection) against constant mover rows [1s, 1s, one-hots].
The cumsum is then ONE masked segmented scan per [128, 512] block:
    state = (mask[t] * state) + dp[t]      (mask 0 at block starts)

Trig: range-reduce with the fp32 magic-round trick; sin = Sin(s*rs);
cos = Sin(-s*|rs| + s/4) (valid since cos(2pi rs) = sin(2pi(1/4 - |rs|))).
Rotation runs in fp16 (2x DVE mode), split across DVE and GpSimd.

Per-engine steady state per half-block: PE 17 matmuls, DVE scan+rs+3 TT,
Scalar 4 ACTs, GpSimd 3 TT. Stages are software-pipelined (manual skew) so
the in-order engine queues never stall on same-iteration dependencies.
DMA: x in on the sync DGE queue, weights/consts on scalar's, outputs on
gpsimd's; all tensors are host-pre-tiled so every DMA is one dense block.
"""

import sys

if "/opt/trn_rl_repo" not in sys.path:
    sys.path.insert(0, "/opt/trn_rl_repo")

from contextlib import ExitStack

import ml_dtypes
import numpy as np

import concourse.bacc as bacc
import concourse.bass as bass
import concourse.mybir as mybir
import concourse.tile as tile
from concourse.bass_utils import run_bass_kernel_spmd

F32 = mybir.dt.float32
F16 = mybir.dt.float16
BF16 = mybir.dt.bfloat16
ADD = mybir.AluOpType.add
SUB = mybir.AluOpType.subtract
MULT = mybir.AluOpType.mult
IDENT = mybir.ActivationFunctionType.Identity
SIN = mybir.ActivationFunctionType.Sin
ABS = mybir.ActivationFunctionType.Abs

D = 2048          # input feature dim (contraction)
P = 256           # delta-pairs dim
ROT = 2 * P       # rotated columns (512)
TL = 2048         # time steps per shard
TB = 512          # time block (one PSUM bank at fp32)
SB = 64           # scan block (base injection granularity)
NS = TB // SB     # scan blocks per time block (8)
NT = TL // TB     # time blocks per shard (4)
NBK = TL // SB    # scan blocks per shard (32)
KC = D // 128     # contraction chunks (16)
NR = 2 + 2 * NS   # setup-matmul stationary rows (18)
N_CORES = 8

MAGIC = 12582912.0          # 1.5 * 2**23: fp32 round-to-int magic constant
SCALE_2PI = 6.28310         # slightly < 2*pi so Sin args stay inside [-pi, pi]
COS_BIAS = SCALE_2PI / 4.0  # cos(2pi y) = sin(SCALE*(1/4 - |wrap(y)|))
NP_BF16 = np.dtype(ml_dtypes.bfloat16)


def build_program(tl: int = TL) -> bass.Bass:
    nt = tl // TB
    nit = nt * 2          # half-block iterations (8)
    nc = bacc.Bacc("TRN2", target_bir_lowering=False, debug=False)

    # Host-pre-tiled inputs: every DMA below reads one dense DRAM block.
    # xf row block tb: [128, KC*TB] fp16 (d-chunks along the free dim)
    xf = nc.dram_tensor("xf", [nt * 128, KC * TB], F16,
                        kind="ExternalInput").ap()
    wh = nc.dram_tensor("wh", [128, KC * P], F16, kind="ExternalInput").ap()
    # setup-matmul stationary rows per (tb, h): [NR, nit*128] bf16
    ext = nc.dram_tensor("ext", [NR, nit * 128], BF16,
                         kind="ExternalInput").ap()
    # constant mover rows: [NR, TB] bf16 (1s, 1s, one-hot pairs)
    mov = nc.dram_tensor("mov", [NR, TB], BF16, kind="ExternalInput").ap()
    # scan mask: [128, TB] fp16, 0.0 at block-start columns
    msk = nc.dram_tensor("msk", [128, TB], F16, kind="ExternalInput").ap()
    # out row block tb: [128, 4*TB] fp16 (quadrants o1h0|o1h1|o2h0|o2h1)
    outT = nc.dram_tensor("outT", [nt * 128, 4 * TB], F16,
                          kind="ExternalOutput").ap()

    with tile.TileContext(nc) as tc, ExitStack() as ctx:
        const_pool = ctx.enter_context(tc.tile_pool(name="const", bufs=1))
        x_pool = ctx.enter_context(tc.tile_pool(name="x", bufs=3))
        psum_pool = ctx.enter_context(tc.tile_pool(name="psum", bufs=4, space="PSUM"))
        ang_pool = ctx.enter_context(tc.tile_pool(name="ang", bufs=3))
        as_pool = ctx.enter_context(tc.tile_pool(name="as", bufs=3))
        rs_pool = ctx.enter_context(tc.tile_pool(name="rs", bufs=2))
        ab_pool = ctx.enter_context(tc.tile_pool(name="ab", bufs=2))
        trig_pool = ctx.enter_context(tc.tile_pool(name="trig", bufs=3))
        rot_pool = ctx.enter_context(tc.tile_pool(name="rot", bufs=2))
        out_pool = ctx.enter_context(tc.tile_pool(name="out", bufs=2))

        # Constants + weights (stationary), on the scalar DGE queue
        msk_sb = const_pool.tile([128, TB], F16, tag="msk")
        nc.scalar.dma_start(msk_sb[:], msk[:])
        mov_sb = const_pool.tile([NR, TB], BF16, tag="mov")
        nc.scalar.dma_start(mov_sb[:], mov[:])
        ext_sb = const_pool.tile([NR, nit * 128], BF16, tag="ext")
        nc.scalar.dma_start(ext_sb[:], ext[:])
        wh_sb = const_pool.tile([128, KC * P], F16, tag="wh")
        nc.scalar.dma_start(wh_sb[:], wh[:])
        magic_sb = const_pool.tile([128, 1], F32, tag="magic")
        nc.gpsimd.memset(magic_sb[:], MAGIC)
        cosb_sb = const_pool.tile([128, 1], F32, tag="cosb")
        nc.gpsimd.memset(cosb_sb[:], COS_BIAS)

        # ---- software-pipelined stages -------------------------------
        xall = {}     # per tb
        oall = {}     # per tb
        dp = {}       # per iteration i = tb*2 + h
        ang = {}
        a_s = {}
        rs = {}
        ab = {}
        sin16 = {}
        cos16 = {}

        def st_dma_in(tb):
            xall[tb] = x_pool.tile([128, KC * TB], F16, tag="xall", name="xall")
            half = KC * TB // 2
            nc.sync.dma_start(xall[tb][:, 0:half],
                              xf[tb * 128:(tb + 1) * 128, 0:half])
            nc.sync.dma_start(xall[tb][:, half:2 * half],
                              xf[tb * 128:(tb + 1) * 128, half:2 * half])

        def st_matmul(i):
            tb, h = i // 2, i % 2
            dpi = psum_pool.tile([128, TB], F32, tag="dp", name="dp")
            dp[i] = dpi
            nc.tensor.matmul(dpi[:], ext_sb[:, i * 128:(i + 1) * 128],
                             mov_sb[:], start=True, stop=False)
            for d in range(KC):
                ws = slice(d * P + h * 128, d * P + (h + 1) * 128)
                xs = slice(d * TB, (d + 1) * TB)
                nc.tensor.matmul(dpi[:], wh_sb[:, ws], xall[tb][:, xs],
                                 start=False, stop=(d == KC - 1))

        def st_scan(i):
            # masked segmented scan: state = mask[t]*state + dp[t]
            ang[i] = ang_pool.tile([128, TB], F32, tag="ang", name="ang")
            nc.vector.tensor_tensor_scan(ang[i][:], msk_sb[:], dp[i][:],
                                         initial=0.0, op0=MULT, op1=ADD)
            del dp[i]
            # a_s = MAGIC - round(ang)  (fp32 magic rounding)
            a_s[i] = as_pool.tile([128, TB], F32, tag="a_s", name="a_s")
            nc.scalar.activation(a_s[i][:], ang[i][:], IDENT,
                                 bias=magic_sb[:], scale=-1.0)

        def st_trig(i):
            # rs = ang - round(ang) in [-0.5, 0.5]
            rs[i] = rs_pool.tile([128, TB], F32, tag="rs", name="rs")
            nc.vector.scalar_tensor_tensor(rs[i][:], a_s[i][:], MAGIC,
                                           ang[i][:], op0=SUB, op1=ADD)
            del a_s[i], ang[i]
            sin16[i] = trig_pool.tile([128, TB], F16, tag="sin", name="sin16")
            nc.scalar.activation(sin16[i][:], rs[i][:], SIN, scale=SCALE_2PI)
            ab[i] = ab_pool.tile([128, TB], F32, tag="ab", name="ab")
            nc.scalar.activation(ab[i][:], rs[i][:], ABS)
            cos16[i] = trig_pool.tile([128, TB], F16, tag="cos", name="cos16")
            nc.scalar.activation(cos16[i][:], ab[i][:], SIN,
                                 scale=-SCALE_2PI, bias=cosb_sb[:])
            del rs[i], ab[i]

        def st_rot(i):
            tb, h = i // 2, i % 2
            if h == 0:
                oall[tb] = out_pool.tile([128, 4 * TB], F16, tag="oall", name="oall")
            x1s = xall[tb][:, h * TB:(h + 1) * TB]
            x2s = xall[tb][:, (2 + h) * TB:(3 + h) * TB]
            st, ct = sin16[i], cos16[i]
            t1 = rot_pool.tile([128, TB], F16, tag="t1")
            nc.vector.tensor_mul(t1[:], x1s, ct[:])
            t2 = rot_pool.tile([128, TB], F16, tag="t2")
            nc.vector.tensor_mul(t2[:], x2s, st[:])
            o1 = oall[tb][:, h * TB:(h + 1) * TB]
            nc.vector.tensor_sub(o1, t1[:], t2[:])
            t3 = rot_pool.tile([128, TB], F16, tag="t3")
            nc.gpsimd.tensor_mul(t3[:], x2s, ct[:])
            t4 = rot_pool.tile([128, TB], F16, tag="t4")
            nc.gpsimd.tensor_mul(t4[:], x1s, st[:])
            o2 = oall[tb][:, (2 + h) * TB:(3 + h) * TB]
            nc.gpsimd.tensor_add(o2, t3[:], t4[:])
            del sin16[i], cos16[i]
            if h == 1:
                nc.gpsimd.dma_start(outT[tb * 128:(tb + 1) * 128, :],
                                    oall[tb][:])
                del xall[tb], oall[tb]

        # prime the x DMA pipeline two blocks deep, then run skewed stages
        st_dma_in(0)
        st_dma_in(1)
        for r in range(nit + 3):
            if r < nit:
                if r % 2 == 1 and r // 2 + 2 < nt:
                    st_dma_in(r // 2 + 2)
                st_matmul(r)
            if 0 <= r - 1 < nit:
                st_scan(r - 1)
            if 0 <= r - 2 < nit:
                st_trig(r - 2)
            if 0 <= r - 3 < nit:
                st_rot(r - 3)

    nc.compile()
    return nc


_NC_CACHE: dict = {}


def _get_nc():
    if "nc" not in _NC_CACHE:
        _NC_CACHE["nc"] = build_program()
    return _NC_CACHE["nc"]


def _tile_x(xt16: np.ndarray, nt: int) -> np.ndarray:
    """[D, tl] fp16 -> [nt*128, KC*TB]: row block tb, d-chunks along free."""
    tl = xt16.shape[1]
    a = xt16.reshape(KC, 128, tl // TB, TB).transpose(2, 1, 0, 3)
    return np.ascontiguousarray(a.reshape((tl // TB) * 128, KC * TB))


def _split_bf16(v: np.ndarray):
    hi = v.astype(NP_BF16)
    lo = (v - hi.astype(np.float64)).astype(NP_BF16)
    return hi, lo


def prepare_weights(W: np.ndarray, b: np.ndarray):
    inv2pi = 1.0 / (2.0 * np.pi)
    Wt = W.astype(np.float64).T * inv2pi                           # [D, P]
    bt = b.astype(np.float64) * inv2pi                             # [P]
    whf = Wt.astype(np.float16)
    # [D, P] -> [128, KC*P] with d-chunks along free dim
    wh_in = np.ascontiguousarray(
        whf.reshape(KC, 128, P).transpose(1, 0, 2).reshape(128, KC * P))
    return wh_in, Wt, bt


def make_in_maps(x: np.ndarray, W: np.ndarray, b: np.ndarray):
    B = x.shape[0]
    wh_in, Wt, bt = prepare_weights(W, b)

    # fp64 cumulative angle at every SB-step boundary, per batch (in turns),
    # computed from the TRUE weights so the W-quantization error is also a
    # <=SB-step random walk. Wrapped mod 1 to keep scan values small.
    T = x.shape[1]
    nblk = T // SB
    xblk = x.reshape(B, nblk, SB, D).sum(axis=2, dtype=np.float64)  # [B, nblk, D]
    dblk = xblk @ Wt + SB * bt                                      # [B, nblk, P]
    bases = np.zeros((B, nblk, P))
    np.cumsum(dblk[:, :-1], axis=1, out=bases[:, 1:])               # exclusive
    bases -= np.round(bases)

    b_hi, b_lo = _split_bf16(bt)

    # constant mover rows [NR, TB]: 1s, 1s, then one-hot pairs at k*SB
    mov_in = np.zeros((NR, TB), NP_BF16)
    mov_in[0] = 1.0
    mov_in[1] = 1.0
    for k in range(NS):
        mov_in[2 + 2 * k, k * SB] = 1.0
        mov_in[3 + 2 * k, k * SB] = 1.0

    msk_in = np.ones((128, TB), np.float16)
    msk_in[:, 0::SB] = 0.0

    in_maps = []
    for c in range(N_CORES):
        bb, hh = c // 2, c % 2
        xt16 = x[bb, hh * TL:(hh + 1) * TL, :].T.astype(np.float16)
        bs = bases[bb, hh * NBK:(hh + 1) * NBK]                     # [NBK, P]
        # ext rows per (tb, h): [NR, nit*128]
        ext_in = np.zeros((NR, NT * 2 * 128), NP_BF16)
        for tb in range(NT):
            for h in range(2):
                i = tb * 2 + h
                cols = slice(i * 128, (i + 1) * 128)
                ps = slice(h * 128, (h + 1) * 128)
                ext_in[0, cols] = b_hi[ps]
                ext_in[1, cols] = b_lo[ps]
                for k in range(NS):
                    bhi, blo = _split_bf16(bs[tb * NS + k, ps])
                    ext_in[2 + 2 * k, cols] = bhi
                    ext_in[3 + 2 * k, cols] = blo
        in_maps.append({
            "xf": _tile_x(xt16, NT),
            "wh": wh_in,
            "ext": np.ascontiguousarray(ext_in),
            "mov": mov_in,
            "msk": msk_in,
        })
    return in_maps


def assemble_output(x: np.ndarray, results) -> np.ndarray:
    B, T, Din = x.shape
    out = np.empty((B, T, Din), np.float32)
    out[:, :, ROT:] = x[:, :, ROT:]
    for c in range(N_CORES):
        bb, hh = c // 2, c % 2
        r = results[c]["outT"].astype(np.float32).reshape(NT, 128, 4, TB)
        # [tb, pp, q(oi,h), u] -> [t_local(tb,u), p(oi,h,pp)]
        blk = r.transpose(0, 3, 2, 1).reshape(TL, ROT)
        out[bb, hh * TL:(hh + 1) * TL, :ROT] = blk
    return out


def kernel(x: np.ndarray, W: np.ndarray, b: np.ndarray) -> np.ndarray:
    nc = _get_nc()
    in_maps = make_in_maps(x, W, b)
    res = run_bass_kernel_spmd(nc, in_maps, list(range(N_CORES)))
    return assemble_output(x, res.results)


# revision 6
# speedup vs baseline: 1.5993x; 1.3099x over previous
"""DD-RoPE kernel for 8x TRN2 NeuronCores.

Reference computation (B=4, T=4096, D=2048, P=256):
    deltas = einsum('btd,pd->btp', x, W) + b     # (B, T, P)
    angles = cumsum(deltas, axis=1)
    out = concat([x1*cos(a) - x2*sin(a), x2*cos(a) + x1*sin(a), x[..., 512:]], -1)

Sharding: 8 shards = 4 batches x 2 T-halves (2048 each), data-parallel.

Numerics: single fp16 matmul pass (W fp16, x fp16, fp32 PSUM). Host sends the
exact fp64 cumulative angle (from the TRUE weights) at every 64-step boundary,
so all quantization error is a <=64-step random walk (sim rel-err ~5e-3 vs
2e-2 budget). Bias + per-block bases enter through one extra "setup" matmul
per time block: stationary rows [b_hi, b_lo, (base_hi_k, base_lo_k)*] (bf16
pairs = exact f32 injection) against constant mover rows [1s, 1s, one-hots].
The cumsum is then ONE masked segmented scan per [128, 512] block:
    state = (mask[t] * state) + dp[t]      (mask 0 at block starts)

Trig: range-reduce with the fp32 magic-round trick; sin = Sin(s*rs);
cos = Sin(-s*|rs| + s/4) (valid since cos(2pi rs) = sin(2pi(1/4 - |rs|))).
Rotation runs in fp16 (2x DVE mode), split across DVE and GpSimd.

Per-engine steady state per half-block: PE 17 matmuls, DVE scan+rs+3 TT,
Scalar 4 ACTs, GpSimd 3 TT. Stages are software-pipelined (manual skew) so
the in-order engine queues never stall on same-iteration dependencies.
DMA: x in on the sync DGE queue, weights/consts on scalar's, outputs on
gpsimd's; all tensors are host-pre-tiled so every DMA is one dense block.
"""

import sys

if "/opt/trn_rl_repo" not in sys.path:
    sys.path.insert(0, "/opt/trn_rl_repo")

from contextlib import ExitStack

import ml_dtypes
import numpy as np

import concourse.bacc as bacc
import concourse.bass as bass
import concourse.mybir as mybir
import concourse.tile as tile
from concourse.bass_utils import run_bass_kernel_spmd

F32 = mybir.dt.float32
F16 = mybir.dt.float16
BF16 = mybir.dt.bfloat16
ADD = mybir.AluOpType.add
SUB = mybir.AluOpType.subtract
MULT = mybir.AluOpType.mult
IDENT = mybir.ActivationFunctionType.Identity
SIN = mybir.ActivationFunctionType.Sin
ABS = mybir.ActivationFunctionType.Abs

D = 2048          # input feature dim (contraction)
P = 256           # delta-pairs dim
ROT = 2 * P       # rotated columns (512)
TL = 2048         # time steps per shard
TB = 512          # time block (one PSUM bank at fp32)
SB = 64           # scan block (base injection granularity)
NS = TB // SB     # scan blocks per time block (8)
NT = TL // TB     # time blocks per shard (4)
NBK = TL // SB    # scan blocks per shard (32)
KC = D // 128     # contraction chunks (16)
NR = 2 + 2 * NS   # setup-matmul stationary rows (18)
N_CORES = 8

MAGIC = 12582912.0          # 1.5 * 2**23: fp32 round-to-int magic constant
SCALE_2PI = 6.28310         # slightly < 2*pi so Sin args stay inside [-pi, pi]
COS_BIAS = SCALE_2PI / 4.0  # cos(2pi y) = sin(SCALE*(1/4 - |wrap(y)|))
NP_BF16 = np.dtype(ml_dtypes.bfloat16)


def build_program(tl: int = TL) -> bass.Bass:
    nt = tl // TB
    nit = nt * 2          # half-block iterations (8)
    nc = bacc.Bacc("TRN2", target_bir_lowering=False, debug=False)

    # Host-pre-tiled inputs: every DMA below reads one dense DRAM block.
    # xf row block tb: [128, KC*TB] fp16 (d-chunks along the free dim)
    xf = nc.dram_tensor("xf", [nt * 128, KC * TB], F16,
                        kind="ExternalInput").ap()
    wh = nc.dram_tensor("wh", [128, KC * P], F16, kind="ExternalInput").ap()
    # setup-matmul stationary rows per (tb, h): [NR, nit*128] bf16
    ext = nc.dram_tensor("ext", [NR, nit * 128], BF16,
                         kind="ExternalInput").ap()
    # constant mover rows: [NR, TB] bf16 (1s, 1s, one-hot pairs)
    mov = nc.dram_tensor("mov", [NR, TB], BF16, kind="ExternalInput").ap()
    # scan mask: [128, TB] fp16, 0.0 at block-start columns
    msk = nc.dram_tensor("msk", [128, TB], F16, kind="ExternalInput").ap()
    # out row block tb: [128, 8*TB] fp16 product quadrants
    # (h0: x1c|x2s|x2c|x1s, then h1 same); host does o1=q0-q1, o2=q2+q3
    outT = nc.dram_tensor("outT", [nt * 128, 8 * TB], F16,
                          kind="ExternalOutput").ap()

    with tile.TileContext(nc) as tc, ExitStack() as ctx:
        const_pool = ctx.enter_context(tc.tile_pool(name="const", bufs=1))
        x_pool = ctx.enter_context(tc.tile_pool(name="x", bufs=3))
        psum_pool = ctx.enter_context(tc.tile_pool(name="psum", bufs=4, space="PSUM"))
        ang_pool = ctx.enter_context(tc.tile_pool(name="ang", bufs=3))
        as_pool = ctx.enter_context(tc.tile_pool(name="as", bufs=3))
        rs_pool = ctx.enter_context(tc.tile_pool(name="rs", bufs=2))
        ab_pool = ctx.enter_context(tc.tile_pool(name="ab", bufs=2))
        trig_pool = ctx.enter_context(tc.tile_pool(name="trig", bufs=3))
        out_pool = ctx.enter_context(tc.tile_pool(name="out", bufs=2))

        # Constants + weights (stationary), on the scalar DGE queue
        msk_sb = const_pool.tile([128, TB], F16, tag="msk")
        nc.scalar.dma_start(msk_sb[:], msk[:])
        mov_sb = const_pool.tile([NR, TB], BF16, tag="mov")
        nc.scalar.dma_start(mov_sb[:], mov[:])
        ext_sb = const_pool.tile([NR, nit * 128], BF16, tag="ext")
        nc.scalar.dma_start(ext_sb[:], ext[:])
        wh_sb = const_pool.tile([128, KC * P], F16, tag="wh")
        nc.scalar.dma_start(wh_sb[:], wh[:])
        magic_sb = const_pool.tile([128, 1], F32, tag="magic")
        nc.gpsimd.memset(magic_sb[:], MAGIC)
        cosb_sb = const_pool.tile([128, 1], F32, tag="cosb")
        nc.gpsimd.memset(cosb_sb[:], COS_BIAS)

        # ---- software-pipelined stages -------------------------------
        xall = {}     # per tb
        oall = {}     # per tb
        dp = {}       # per iteration i = tb*2 + h
        ang = {}
        a_s = {}
        rs = {}
        ab = {}
        sin16 = {}
        cos16 = {}

        def st_dma_in(tb):
            xall[tb] = x_pool.tile([128, KC * TB], F16, tag="xall", name="xall")
            q = KC * TB // 4
            for j in range(4):
                nc.sync.dma_start(xall[tb][:, j * q:(j + 1) * q],
                                  xf[tb * 128:(tb + 1) * 128, j * q:(j + 1) * q])

        def st_matmul(i):
            tb, h = i // 2, i % 2
            dpi = psum_pool.tile([128, TB], F32, tag="dp", name="dp")
            dp[i] = dpi
            nc.tensor.matmul(dpi[:], ext_sb[:, i * 128:(i + 1) * 128],
                             mov_sb[:], start=True, stop=False)
            for d in range(KC):
                ws = slice(d * P + h * 128, d * P + (h + 1) * 128)
                xs = slice(d * TB, (d + 1) * TB)
                nc.tensor.matmul(dpi[:], wh_sb[:, ws], xall[tb][:, xs],
                                 start=False, stop=(d == KC - 1))

        def st_scan(i):
            # masked segmented scan: state = mask[t]*state + dp[t]
            ang[i] = ang_pool.tile([128, TB], F32, tag="ang", name="ang")
            nc.vector.tensor_tensor_scan(ang[i][:], msk_sb[:], dp[i][:],
                                         initial=0.0, op0=MULT, op1=ADD)
            del dp[i]
            # a_s = MAGIC - round(ang)  (fp32 magic rounding)
            a_s[i] = as_pool.tile([128, TB], F32, tag="a_s", name="a_s")
            nc.scalar.activation(a_s[i][:], ang[i][:], IDENT,
                                 bias=magic_sb[:], scale=-1.0)

        def st_trig(i):
            # rs = ang - round(ang) in [-0.5, 0.5]
            rs[i] = rs_pool.tile([128, TB], F32, tag="rs", name="rs")
            nc.vector.scalar_tensor_tensor(rs[i][:], a_s[i][:], MAGIC,
                                           ang[i][:], op0=SUB, op1=ADD)
            del a_s[i], ang[i]
            sin16[i] = trig_pool.tile([128, TB], F16, tag="sin", name="sin16")
            nc.scalar.activation(sin16[i][:], rs[i][:], SIN, scale=SCALE_2PI)
            ab[i] = ab_pool.tile([128, TB], F32, tag="ab", name="ab")
            nc.scalar.activation(ab[i][:], rs[i][:], ABS)
            cos16[i] = trig_pool.tile([128, TB], F16, tag="cos", name="cos16")
            nc.scalar.activation(cos16[i][:], ab[i][:], SIN,
                                 scale=-SCALE_2PI, bias=cosb_sb[:])
            del rs[i], ab[i]

        def st_rot(i):
            tb, h = i // 2, i % 2
            if h == 0:
                oall[tb] = out_pool.tile([128, 8 * TB], F16, tag="oall", name="oall")
            x1s = xall[tb][:, h * TB:(h + 1) * TB]
            x2s = xall[tb][:, (2 + h) * TB:(3 + h) * TB]
            st, ct = sin16[i], cos16[i]
            q = lambda j: oall[tb][:, (4 * h + j) * TB:(4 * h + j + 1) * TB]
            nc.vector.tensor_mul(q(0), x1s, ct[:])     # x1*cos
            nc.vector.tensor_mul(q(1), x2s, st[:])     # x2*sin
            nc.gpsimd.tensor_mul(q(2), x2s, ct[:])     # x2*cos
            nc.gpsimd.tensor_mul(q(3), x1s, st[:])     # x1*sin
            del sin16[i], cos16[i]
            if h == 1:
                dma_eng = nc.gpsimd if tb % 2 == 0 else nc.scalar
                dma_eng.dma_start(outT[tb * 128:(tb + 1) * 128, :],
                                  oall[tb][:])
                del xall[tb], oall[tb]

        # prime the x DMA pipeline two blocks deep, then run skewed stages
        st_dma_in(0)
        st_dma_in(1)
        for r in range(nit + 3):
            if r < nit:
                if r % 2 == 1 and r // 2 + 2 < nt:
                    st_dma_in(r // 2 + 2)
                st_matmul(r)
            if 0 <= r - 1 < nit:
                st_scan(r - 1)
            if 0 <= r - 2 < nit:
                st_trig(r - 2)
            if 0 <= r - 3 < nit:
                st_rot(r - 3)

    nc.compile()
    return nc


_NC_CACHE: dict = {}


def _get_nc():
    if "nc" not in _NC_CACHE:
        _NC_CACHE["nc"] = build_program()
    return _NC_CACHE["nc"]


def _tile_x(xt16: np.ndarray, nt: int) -> np.ndarray:
    """[D, tl] fp16 -> [nt*128, KC*TB]: row block tb, d-chunks along free."""
    tl = xt16.shape[1]
    a = xt16.reshape(KC, 128, tl // TB, TB).transpose(2, 1, 0, 3)
    return np.ascontiguousarray(a.reshape((tl // TB) * 128, KC * TB))


def _split_bf16(v: np.ndarray):
    hi = v.astype(NP_BF16)
    lo = (v - hi.astype(np.float64)).astype(NP_BF16)
    return hi, lo


def prepare_weights(W: np.ndarray, b: np.ndarray):
    inv2pi = 1.0 / (2.0 * np.pi)
    Wt = W.astype(np.float64).T * inv2pi                           # [D, P]
    bt = b.astype(np.float64) * inv2pi                             # [P]
    whf = Wt.astype(np.float16)
    # [D, P] -> [128, KC*P] with d-chunks along free dim
    wh_in = np.ascontiguousarray(
        whf.reshape(KC, 128, P).transpose(1, 0, 2).reshape(128, KC * P))
    return wh_in, Wt, bt


def make_in_maps(x: np.ndarray, W: np.ndarray, b: np.ndarray):
    B = x.shape[0]
    wh_in, Wt, bt = prepare_weights(W, b)

    # fp64 cumulative angle at every SB-step boundary, per batch (in turns),
    # computed from the TRUE weights so the W-quantization error is also a
    # <=SB-step random walk. Wrapped mod 1 to keep scan values small.
    T = x.shape[1]
    nblk = T // SB
    xblk = x.reshape(B, nblk, SB, D).sum(axis=2, dtype=np.float64)  # [B, nblk, D]
    dblk = xblk @ Wt + SB * bt                                      # [B, nblk, P]
    bases = np.zeros((B, nblk, P))
    np.cumsum(dblk[:, :-1], axis=1, out=bases[:, 1:])               # exclusive
    bases -= np.round(bases)

    b_hi, b_lo = _split_bf16(bt)

    # constant mover rows [NR, TB]: 1s, 1s, then one-hot pairs at k*SB
    mov_in = np.zeros((NR, TB), NP_BF16)
    mov_in[0] = 1.0
    mov_in[1] = 1.0
    for k in range(NS):
        mov_in[2 + 2 * k, k * SB] = 1.0
        mov_in[3 + 2 * k, k * SB] = 1.0

    msk_in = np.ones((128, TB), np.float16)
    msk_in[:, 0::SB] = 0.0

    in_maps = []
    for c in range(N_CORES):
        bb, hh = c // 2, c % 2
        xt16 = x[bb, hh * TL:(hh + 1) * TL, :].T.astype(np.float16)
        bs = bases[bb, hh * NBK:(hh + 1) * NBK]                     # [NBK, P]
        # ext rows per (tb, h): [NR, nit*128]
        ext_in = np.zeros((NR, NT * 2 * 128), NP_BF16)
        for tb in range(NT):
            for h in range(2):
                i = tb * 2 + h
                cols = slice(i * 128, (i + 1) * 128)
                ps = slice(h * 128, (h + 1) * 128)
                ext_in[0, cols] = b_hi[ps]
                ext_in[1, cols] = b_lo[ps]
                for k in range(NS):
                    bhi, blo = _split_bf16(bs[tb * NS + k, ps])
                    ext_in[2 + 2 * k, cols] = bhi
                    ext_in[3 + 2 * k, cols] = blo
        in_maps.append({
            "xf": _tile_x(xt16, NT),
            "wh": wh_in,
            "ext": np.ascontiguousarray(ext_in),
            "mov": mov_in,
            "msk": msk_in,
        })
    return in_maps


def assemble_output(x: np.ndarray, results) -> np.ndarray:
    B, T, Din = x.shape
    out = np.empty((B, T, Din), np.float32)
    out[:, :, ROT:] = x[:, :, ROT:]
    for c in range(N_CORES):
        bb, hh = c // 2, c % 2
        r = results[c]["outT"].astype(np.float32).reshape(NT, 128, 2, 4, TB)
        o1 = r[:, :, :, 0] - r[:, :, :, 1]            # [tb, pp, h, u]
        o2 = r[:, :, :, 2] + r[:, :, :, 3]
        q = np.stack([o1, o2], axis=2)                # [tb, pp, oi, h, u]
        # [tb, pp, oi, h, u] -> [t_local(tb,u), p(oi,h,pp)]
        blk = q.reshape(NT, 128, 4, TB).transpose(0, 3, 2, 1).reshape(TL, ROT)
        out[bb, hh * TL:(hh + 1) * TL, :ROT] = blk
    return out


def kernel(x: np.ndarray, W: np.ndarray, b: np.ndarray) -> np.ndarray:
    nc = _get_nc()
    in_maps = make_in_maps(x, W, b)
    res = run_bass_kernel_spmd(nc, in_maps, list(range(N_CORES)))
    return assemble_output(x, res.results)


# revision 7
# speedup vs baseline: 1.6380x; 1.0242x over previous
"""DD-RoPE kernel for 8x TRN2 NeuronCores.

Reference computation (B=4, T=4096, D=2048, P=256):
    deltas = einsum('btd,pd->btp', x, W) + b     # (B, T, P)
    angles = cumsum(deltas, axis=1)
    out = concat([x1*cos(a) - x2*sin(a), x2*cos(a) + x1*sin(a), x[..., 512:]], -1)

Sharding: 8 shards = 4 batches x 2 T-halves (2048 each), data-parallel.

Numerics: single fp16 matmul pass (W fp16, x fp16, fp32 PSUM). Host sends the
exact fp64 cumulative angle (from the TRUE weights) at every 64-step boundary,
so all quantization error is a <=64-step random walk (sim rel-err ~5e-3 vs
2e-2 budget). Bias + per-block bases enter through one extra "setup" matmul
per time block: stationary rows [b_hi, b_lo, (base_hi_k, base_lo_k)*] (bf16
pairs = exact f32 injection) against constant mover rows [1s, 1s, one-hots].
The cumsum is then ONE masked segmented scan per [128, 512] block:
    state = (mask[t] * state) + dp[t]      (mask 0 at block starts)

Trig: range-reduce with the fp32 magic-round trick; sin = Sin(s*rs);
cos = Sin(-s*|rs| + s/4) (valid since cos(2pi rs) = sin(2pi(1/4 - |rs|))).
Rotation runs in fp16 (2x DVE mode), split across DVE and GpSimd.

Per-engine steady state per half-block: PE 17 matmuls, DVE scan+rs+3 TT,
Scalar 4 ACTs, GpSimd 3 TT. Stages are software-pipelined (manual skew) so
the in-order engine queues never stall on same-iteration dependencies.
DMA: x in on the sync DGE queue, weights/consts on scalar's, outputs on
gpsimd's; all tensors are host-pre-tiled so every DMA is one dense block.
"""

import sys

if "/opt/trn_rl_repo" not in sys.path:
    sys.path.insert(0, "/opt/trn_rl_repo")

from contextlib import ExitStack

import ml_dtypes
import numpy as np

import concourse.bacc as bacc
import concourse.bass as bass
import concourse.mybir as mybir
import concourse.tile as tile
from concourse.bass_utils import run_bass_kernel_spmd

F32 = mybir.dt.float32
F16 = mybir.dt.float16
BF16 = mybir.dt.bfloat16
ADD = mybir.AluOpType.add
SUB = mybir.AluOpType.subtract
MULT = mybir.AluOpType.mult
IDENT = mybir.ActivationFunctionType.Identity
SIN = mybir.ActivationFunctionType.Sin
ABS = mybir.ActivationFunctionType.Abs

D = 2048          # input feature dim (contraction)
P = 256           # delta-pairs dim
ROT = 2 * P       # rotated columns (512)
TL = 2048         # time steps per shard
TB = 512          # time block (one PSUM bank at fp32)
SB = 64           # scan block (base injection granularity)
NS = TB // SB     # scan blocks per time block (8)
NT = TL // TB     # time blocks per shard (4)
NBK = TL // SB    # scan blocks per shard (32)
KC = D // 128     # contraction chunks (16)
NR = 2 + 2 * NS   # setup-matmul stationary rows (18)
N_CORES = 8

MAGIC = 12582912.0          # 1.5 * 2**23: fp32 round-to-int magic constant
SCALE_2PI = 6.28310         # slightly < 2*pi so Sin args stay inside [-pi, pi]
COS_BIAS = SCALE_2PI / 4.0  # cos(2pi y) = sin(SCALE*(1/4 - |wrap(y)|))
NP_BF16 = np.dtype(ml_dtypes.bfloat16)


def build_program(tl: int = TL) -> bass.Bass:
    nt = tl // TB
    nit = nt * 2          # half-block iterations (8)
    nc = bacc.Bacc("TRN2", target_bir_lowering=False, debug=False)

    # Host-pre-tiled inputs: every DMA below reads one dense DRAM block.
    # xf row block tb: [128, KC*TB] fp16 (d-chunks along the free dim)
    xf = nc.dram_tensor("xf", [nt * 128, KC * TB], F16,
                        kind="ExternalInput").ap()
    wh = nc.dram_tensor("wh", [128, KC * P], F16, kind="ExternalInput").ap()
    # setup-matmul stationary rows per (tb, h): [NR, nit*128] bf16
    ext = nc.dram_tensor("ext", [NR, nit * 128], BF16,
                         kind="ExternalInput").ap()
    # constant mover rows: [NR, TB] bf16 (1s, 1s, one-hot pairs)
    mov = nc.dram_tensor("mov", [NR, TB], BF16, kind="ExternalInput").ap()
    # scan mask: [128, TB] fp16, 0.0 at block-start columns
    msk = nc.dram_tensor("msk", [128, TB], F16, kind="ExternalInput").ap()
    # out row block tb: [128, 8*TB] fp16 product quadrants
    # (h0: x1c|x2s|x2c|x1s, then h1 same); host does o1=q0-q1, o2=q2+q3
    outT = nc.dram_tensor("outT", [nt * 128, 8 * TB], F16,
                          kind="ExternalOutput").ap()

    with tile.TileContext(nc) as tc, ExitStack() as ctx:
        const_pool = ctx.enter_context(tc.tile_pool(name="const", bufs=1))
        x_pool = ctx.enter_context(tc.tile_pool(name="x", bufs=3))
        psum_pool = ctx.enter_context(tc.tile_pool(name="psum", bufs=4, space="PSUM"))
        ang_pool = ctx.enter_context(tc.tile_pool(name="ang", bufs=3))
        as_pool = ctx.enter_context(tc.tile_pool(name="as", bufs=3))
        rs_pool = ctx.enter_context(tc.tile_pool(name="rs", bufs=2))
        ab_pool = ctx.enter_context(tc.tile_pool(name="ab", bufs=2))
        trig_pool = ctx.enter_context(tc.tile_pool(name="trig", bufs=3))
        out_pool = ctx.enter_context(tc.tile_pool(name="out", bufs=2))

        # Constants + weights: tiny ext/mov lead the sync queue (they gate
        # the setup matmul); wh leads the scalar queue (gates weight
        # matmuls); msk follows (needed only once scans start)
        ext_sb = const_pool.tile([NR, nit * 128], BF16, tag="ext")
        nc.sync.dma_start(ext_sb[:], ext[:])
        mov_sb = const_pool.tile([NR, TB], BF16, tag="mov")
        nc.sync.dma_start(mov_sb[:], mov[:])
        wh_sb = const_pool.tile([128, KC * P], F16, tag="wh")
        nc.scalar.dma_start(wh_sb[:], wh[:])
        msk_sb = const_pool.tile([128, TB], F16, tag="msk")
        nc.scalar.dma_start(msk_sb[:], msk[:])
        magic_sb = const_pool.tile([128, 1], F32, tag="magic")
        nc.gpsimd.memset(magic_sb[:], MAGIC)
        cosb_sb = const_pool.tile([128, 1], F32, tag="cosb")
        nc.gpsimd.memset(cosb_sb[:], COS_BIAS)

        # ---- software-pipelined stages -------------------------------
        xall = {}     # per tb
        oall = {}     # per tb
        dp = {}       # per iteration i = tb*2 + h
        ang = {}
        a_s = {}
        rs = {}
        ab = {}
        sin16 = {}
        cos16 = {}

        def st_dma_in(tb):
            xall[tb] = x_pool.tile([128, KC * TB], F16, tag="xall", name="xall")
            q = KC * TB // 4
            for j in range(4):
                nc.sync.dma_start(xall[tb][:, j * q:(j + 1) * q],
                                  xf[tb * 128:(tb + 1) * 128, j * q:(j + 1) * q])

        def st_matmul(i):
            tb, h = i // 2, i % 2
            dpi = psum_pool.tile([128, TB], F32, tag="dp", name="dp")
            dp[i] = dpi
            nc.tensor.matmul(dpi[:], ext_sb[:, i * 128:(i + 1) * 128],
                             mov_sb[:], start=True, stop=False)
            for d in range(KC):
                ws = slice(d * P + h * 128, d * P + (h + 1) * 128)
                xs = slice(d * TB, (d + 1) * TB)
                nc.tensor.matmul(dpi[:], wh_sb[:, ws], xall[tb][:, xs],
                                 start=False, stop=(d == KC - 1))

        def st_scan(i):
            # masked segmented scan: state = mask[t]*state + dp[t]
            ang[i] = ang_pool.tile([128, TB], F32, tag="ang", name="ang")
            nc.vector.tensor_tensor_scan(ang[i][:], msk_sb[:], dp[i][:],
                                         initial=0.0, op0=MULT, op1=ADD)
            del dp[i]
            # a_s = MAGIC - round(ang)  (fp32 magic rounding)
            a_s[i] = as_pool.tile([128, TB], F32, tag="a_s", name="a_s")
            nc.scalar.activation(a_s[i][:], ang[i][:], IDENT,
                                 bias=magic_sb[:], scale=-1.0)

        def st_trig(i):
            # rs = ang - round(ang) in [-0.5, 0.5]
            rs[i] = rs_pool.tile([128, TB], F32, tag="rs", name="rs")
            nc.vector.scalar_tensor_tensor(rs[i][:], a_s[i][:], MAGIC,
                                           ang[i][:], op0=SUB, op1=ADD)
            del a_s[i], ang[i]
            sin16[i] = trig_pool.tile([128, TB], F16, tag="sin", name="sin16")
            nc.scalar.activation(sin16[i][:], rs[i][:], SIN, scale=SCALE_2PI)
            ab[i] = ab_pool.tile([128, TB], F32, tag="ab", name="ab")
            nc.scalar.activation(ab[i][:], rs[i][:], ABS)
            cos16[i] = trig_pool.tile([128, TB], F16, tag="cos", name="cos16")
            nc.scalar.activation(cos16[i][:], ab[i][:], SIN,
                                 scale=-SCALE_2PI, bias=cosb_sb[:])
            del rs[i], ab[i]

        def st_rot(i):
            tb, h = i // 2, i % 2
            if h == 0:
                oall[tb] = out_pool.tile([128, 8 * TB], F16, tag="oall", name="oall")
            x1s = xall[tb][:, h * TB:(h + 1) * TB]
            x2s = xall[tb][:, (2 + h) * TB:(3 + h) * TB]
            st, ct = sin16[i], cos16[i]
            q = lambda j: oall[tb][:, (4 * h + j) * TB:(4 * h + j + 1) * TB]
            nc.vector.tensor_mul(q(0), x1s, ct[:])     # x1*cos
            nc.vector.tensor_mul(q(1), x2s, st[:])     # x2*sin
            nc.gpsimd.tensor_mul(q(2), x2s, ct[:])     # x2*cos
            nc.gpsimd.tensor_mul(q(3), x1s, st[:])     # x1*sin
            del sin16[i], cos16[i]
            dma_eng = nc.gpsimd if tb % 2 == 0 else nc.scalar
            cs = slice(4 * h * TB, 4 * (h + 1) * TB)
            dma_eng.dma_start(outT[tb * 128:(tb + 1) * 128, cs],
                              oall[tb][:, cs])
            if h == 1:
                del xall[tb], oall[tb]

        # prime the x DMA pipeline two blocks deep, then run skewed stages
        st_dma_in(0)
        st_dma_in(1)
        for r in range(nit + 3):
            if r < nit:
                if r % 2 == 1 and r // 2 + 2 < nt:
                    st_dma_in(r // 2 + 2)
                st_matmul(r)
            if 0 <= r - 1 < nit:
                st_scan(r - 1)
            if 0 <= r - 2 < nit:
                st_trig(r - 2)
            if 0 <= r - 3 < nit:
                st_rot(r - 3)

    nc.compile()
    return nc


_NC_CACHE: dict = {}


def _get_nc():
    if "nc" not in _NC_CACHE:
        _NC_CACHE["nc"] = build_program()
    return _NC_CACHE["nc"]


def _tile_x(xt16: np.ndarray, nt: int) -> np.ndarray:
    """[D, tl] fp16 -> [nt*128, KC*TB]: row block tb, d-chunks along free."""
    tl = xt16.shape[1]
    a = xt16.reshape(KC, 128, tl // TB, TB).transpose(2, 1, 0, 3)
    return np.ascontiguousarray(a.reshape((tl // TB) * 128, KC * TB))


def _split_bf16(v: np.ndarray):
    hi = v.astype(NP_BF16)
    lo = (v - hi.astype(np.float64)).astype(NP_BF16)
    return hi, lo


def prepare_weights(W: np.ndarray, b: np.ndarray):
    inv2pi = 1.0 / (2.0 * np.pi)
    Wt = W.astype(np.float64).T * inv2pi                           # [D, P]
    bt = b.astype(np.float64) * inv2pi                             # [P]
    whf = Wt.astype(np.float16)
    # [D, P] -> [128, KC*P] with d-chunks along free dim
    wh_in = np.ascontiguousarray(
        whf.reshape(KC, 128, P).transpose(1, 0, 2).reshape(128, KC * P))
    return wh_in, Wt, bt


def make_in_maps(x: np.ndarray, W: np.ndarray, b: np.ndarray):
    B = x.shape[0]
    wh_in, Wt, bt = prepare_weights(W, b)

    # fp64 cumulative angle at every SB-step boundary, per batch (in turns),
    # computed from the TRUE weights so the W-quantization error is also a
    # <=SB-step random walk. Wrapped mod 1 to keep scan values small.
    T = x.shape[1]
    nblk = T // SB
    xblk = x.reshape(B, nblk, SB, D).sum(axis=2, dtype=np.float64)  # [B, nblk, D]
    dblk = xblk @ Wt + SB * bt                                      # [B, nblk, P]
    bases = np.zeros((B, nblk, P))
    np.cumsum(dblk[:, :-1], axis=1, out=bases[:, 1:])               # exclusive
    bases -= np.round(bases)

    b_hi, b_lo = _split_bf16(bt)

    # constant mover rows [NR, TB]: 1s, 1s, then one-hot pairs at k*SB
    mov_in = np.zeros((NR, TB), NP_BF16)
    mov_in[0] = 1.0
    mov_in[1] = 1.0
    for k in range(NS):
        mov_in[2 + 2 * k, k * SB] = 1.0
        mov_in[3 + 2 * k, k * SB] = 1.0

    msk_in = np.ones((128, TB), np.float16)
    msk_in[:, 0::SB] = 0.0

    in_maps = []
    for c in range(N_CORES):
        bb, hh = c // 2, c % 2
        xt16 = x[bb, hh * TL:(hh + 1) * TL, :].T.astype(np.float16)
        bs = bases[bb, hh * NBK:(hh + 1) * NBK]                     # [NBK, P]
        # ext rows per (tb, h): [NR, nit*128]
        ext_in = np.zeros((NR, NT * 2 * 128), NP_BF16)
        for tb in range(NT):
            for h in range(2):
                i = tb * 2 + h
                cols = slice(i * 128, (i + 1) * 128)
                ps = slice(h * 128, (h + 1) * 128)
                ext_in[0, cols] = b_hi[ps]
                ext_in[1, cols] = b_lo[ps]
                for k in range(NS):
                    bhi, blo = _split_bf16(bs[tb * NS + k, ps])
                    ext_in[2 + 2 * k, cols] = bhi
                    ext_in[3 + 2 * k, cols] = blo
        in_maps.append({
            "xf": _tile_x(xt16, NT),
            "wh": wh_in,
            "ext": np.ascontiguousarray(ext_in),
            "mov": mov_in,
            "msk": msk_in,
        })
    return in_maps


def assemble_output(x: np.ndarray, results) -> np.ndarray:
    B, T, Din = x.shape
    out = np.empty((B, T, Din), np.float32)
    out[:, :, ROT:] = x[:, :, ROT:]
    for c in range(N_CORES):
        bb, hh = c // 2, c % 2
        r = results[c]["outT"].astype(np.float32).reshape(NT, 128, 2, 4, TB)
        o1 = r[:, :, :, 0] - r[:, :, :, 1]            # [tb, pp, h, u]
        o2 = r[:, :, :, 2] + r[:, :, :, 3]
        q = np.stack([o1, o2], axis=2)                # [tb, pp, oi, h, u]
        # [tb, pp, oi, h, u] -> [t_local(tb,u), p(oi,h,pp)]
        blk = q.reshape(NT, 128, 4, TB).transpose(0, 3, 2, 1).reshape(TL, ROT)
        out[bb, hh * TL:(hh + 1) * TL, :ROT] = blk
    return out


def kernel(x: np.ndarray, W: np.ndarray, b: np.ndarray) -> np.ndarray:
    nc = _get_nc()
    in_maps = make_in_maps(x, W, b)
    res = run_bass_kernel_spmd(nc, in_maps, list(range(N_CORES)))
    return assemble_output(x, res.results)
